# revision 5
# baseline (speedup 1.0000x reference)
"""AFM sparse-attention kernel for 8 TRN2 NeuronCores.

Problem (per reference):
    value[b,i,j,:] = emb[b,i,:] * emb[b,j,:]                  [B,N,N,d]
    qk = LeakyReLU(value @ w_W.T + w_b, 0.01)
    logits = qk @ a_W (+ a_b, softmax-invariant)
    alphas = softmax(logits, axis=-1)[..., None]              [B,N,N,1]
    returns (alphas, value)

B=256, N=64, d=64.  Pure data parallel: batch sharded 32/core over 8 cores.

Device-side design per core (16 groups of 2 batches):
  - value (the 268MB output) is produced in DMA-native layout
    [i on partitions, (j,d) on free] as:  EF (emb[b] flattened,
    replicated across partitions via a K=2 selector matmul into PSUM)
    multiplied by a stride-0 broadcast view of emb rows (DVE).
    The per-batch store is one fully contiguous 1MB DMA.
  - qk[i,(j,e)] = ET_ext.T @ WJ where ET_ext = [emb[b].T ; ones] and
    WJ[d',(j,e)] = ET_ext[d',j] * wWT2_ext[d',e] (one DVE op per batch
    using stride-0 views; row 64 supplies the w_b bias).
  - LeakyReLU(y)*a = 0.01*a*y + 0.99*a*Relu(y); the linear term is an
    extra matmul (per-column scale c[d]), Relu runs on ACT, and the
    a_W-weighted e-contraction becomes two sign-grouped tensor_reduce
    ops (|a_W|*0.99 folded into the weights host-side).
  - softmax over j on [2*64 partitions, 64] tiles.
"""

import numpy as np

B, N, D = 256, 64, 64
NCORES = 8
BS = B // NCORES          # 32 batches per core
NG = BS // 2              # 16 two-batch groups
NEG_SLOPE = 0.01
F32 = None  # set after mybir import

_CACHE = {}


def _build_nc(pos_cnt):
    import concourse.bass as bass
    import concourse.bacc as bacc
    import concourse.mybir as mybir
    import concourse.tile as tile

    f32 = mybir.dt.float32
    nc = bacc.Bacc()

    embT = nc.declare_dram_parameter("embT", [BS, 65, N], f32, isOutput=False)
    embF = nc.declare_dram_parameter("embF", [BS, N * D], f32, isOutput=False)
    wWT2 = nc.declare_dram_parameter("wWT2", [65, N], f32, isOutput=False)
    wA = nc.declare_dram_parameter("wA", [65, 1], f32, isOutput=False)
    sel = nc.declare_dram_parameter("sel", [2, 128], f32, isOutput=False)
    value_o = nc.declare_dram_parameter("value", [BS, N, N * D], f32, isOutput=True)
    alpha_o = nc.declare_dram_parameter("alphas", [BS * N, N], f32, isOutput=True)

    Pp = pos_cnt  # columns 0..Pp-1 positive a_W, Pp..63 negative

    with tile.TileContext(nc) as tc:
        with (
            tc.tile_pool(name="consts", bufs=1) as consts,
            tc.tile_pool(name="inp", bufs=3) as inp,
            tc.tile_pool(name="wj", bufs=2) as wjp,
            tc.tile_pool(name="big", bufs=2) as big,
            tc.tile_pool(name="lq", bufs=2) as lqp,
            tc.tile_pool(name="sm", bufs=3) as smp,
            tc.tile_pool(name="efp", bufs=2, space="PSUM") as efpp,
            tc.tile_pool(name="qkp", bufs=3, space="PSUM") as qkpp,
            tc.tile_pool(name="ap", bufs=2, space="PSUM") as app,
        ):
            WT = consts.tile([65, N], f32)
            nc.sync.dma_start(out=WT, in_=wWT2[:])
            WA = consts.tile([65, 1], f32)
            nc.sync.dma_start(out=WA, in_=wA[:])
            SEL = consts.tile([2, 128], f32)
            nc.sync.dma_start(out=SEL, in_=sel[:])

            wta = WT[:]
            # viewWT[d', (j, e)] = WT[d', e]   (j outer, stride 0)
            viewWT = bass.AP(tensor=wta.tensor, offset=wta.offset,
                             ap=[wta.ap[0], [0, N], [wta.ap[1][0], N]])

            for g in range(NG):
                b0, b1 = 2 * g, 2 * g + 1
                # ---- loads
                ET0 = inp.tile([65, N], f32, tag="et0")
                nc.sync.dma_start(out=ET0, in_=embT[b0])
                ET1 = inp.tile([65, N], f32, tag="et1")
                nc.sync.dma_start(out=ET1, in_=embT[b1])
                EMB2 = inp.tile([2, N * D], f32, tag="emb2")
                nc.sync.dma_start(out=EMB2, in_=embF[b0:b0 + 2])
                E128 = inp.tile([128, D], f32, tag="e128")
                nc.sync.dma_start(
                    out=E128,
                    in_=embF[b0:b0 + 2].rearrange("b (i d) -> (b i) d", d=D))

                # ---- WJ build (one [65, N*N] DVE op per batch)
                WJ0 = wjp.tile([65, N * N], f32, tag="wj0")
                WJ1 = wjp.tile([65, N * N], f32, tag="wj1")
                for ET, WJ in ((ET0, WJ0), (ET1, WJ1)):
                    eta = ET[:]
                    viewJ = bass.AP(tensor=eta.tensor, offset=eta.offset,
                                    ap=[eta.ap[0], [eta.ap[1][0], N], [0, N]])
                    nc.vector.tensor_mul(WJ, viewJ, viewWT)

                # WJA[d', j] = ET[d', j] * WA[d']  (A-term moving operand)
                WJA0 = inp.tile([65, N], f32, tag="wja0")
                nc.vector.tensor_scalar_mul(WJA0, ET0, WA[:, 0:1])
                WJA1 = inp.tile([65, N], f32, tag="wja1")
                nc.vector.tensor_scalar_mul(WJA1, ET1, WA[:, 0:1])

                # ---- A-term matmuls -> PSUM [128, 64]
                Aps = app.tile([128, N], f32, tag="aps")
                nc.tensor.matmul(Aps[0:64, :], ET0, WJA0, start=True, stop=True)
                nc.tensor.matmul(Aps[64:128, :], ET1, WJA1, start=True, stop=True,
                                 tile_position=(0, 64))

                # ---- value + qk chunk loop (8 chunks of 512)
                V2 = big.tile([128, N * D], f32, tag="v2")
                LQ = lqp.tile([128, N * N], f32, tag="lq")
                e1a = E128[:]
                for c in range(8):
                    sl = slice(512 * c, 512 * (c + 1))
                    # EF chunk: replicate emb2 rows across partition halves
                    EF = efpp.tile([128, 512], f32, tag="ef")
                    nc.tensor.matmul(EF, SEL, EMB2[:, sl], start=True, stop=True)
                    # value chunk: E128 row (stride-0 over j) * EF
                    viewE = bass.AP(
                        tensor=e1a.tensor, offset=e1a.offset,
                        ap=[e1a.ap[0], [0, 8], [e1a.ap[1][0], D]])
                    nc.vector.tensor_mul(V2[:, sl], EF, viewE)
                    # qk chunk for both batches -> one [128, 512] psum tile
                    QK = qkpp.tile([128, 512], f32, tag="qk")
                    nc.tensor.matmul(QK[0:64, :], ET0, WJ0[:, sl],
                                     start=True, stop=True)
                    nc.tensor.matmul(QK[64:128, :], ET1, WJ1[:, sl],
                                     start=True, stop=True, tile_position=(0, 64))
                    nc.scalar.activation(LQ[:, sl], QK,
                                         mybir.ActivationFunctionType.Relu)

                # ---- logits: sign-grouped reduces over e (inner dim)
                LG = smp.tile([128, N], f32, tag="lg")
                lqa = LQ[:]
                fs = lqa.ap[1][0]  # free step of LQ (elements)

                def lq_view(lo, cnt):
                    return bass.AP(tensor=lqa.tensor,
                                   offset=lqa.offset + fs * lo,
                                   ap=[lqa.ap[0], [fs * N, N], [fs, cnt]])

                if Pp > 0 and Pp < N:
                    RP = smp.tile([128, N], f32, tag="rp")
                    nc.vector.tensor_reduce(RP, lq_view(0, Pp),
                                            axis=mybir.AxisListType.X,
                                            op=mybir.AluOpType.add)
                    RN = smp.tile([128, N], f32, tag="rn")
                    nc.vector.tensor_reduce(RN, lq_view(Pp, N - Pp),
                                            axis=mybir.AxisListType.X,
                                            op=mybir.AluOpType.add, negate=True)
                    nc.vector.tensor_add(LG, RP, RN)
                else:
                    nc.vector.tensor_reduce(LG, lq_view(0, N),
                                            axis=mybir.AxisListType.X,
                                            op=mybir.AluOpType.add,
                                            negate=(Pp == 0))
                LG2 = smp.tile([128, N], f32, tag="lg2")
                nc.vector.tensor_add(LG2, LG, Aps)

                # ---- softmax over free dim (j)
                MX = smp.tile([128, 1], f32, tag="mx")
                nc.vector.tensor_reduce(MX, LG2, axis=mybir.AxisListType.X,
                                        op=mybir.AluOpType.max, negate=True)
                EX = smp.tile([128, N], f32, tag="ex")
                nc.scalar.activation(EX, LG2, mybir.ActivationFunctionType.Exp,
                                     bias=MX[:, 0:1], scale=1.0)
                SM = smp.tile([128, 1], f32, tag="sum")
                nc.vector.tensor_reduce(SM, EX, axis=mybir.AxisListType.X,
                                        op=mybir.AluOpType.add)
                RC = smp.tile([128, 1], f32, tag="rc")
                nc.vector.reciprocal(RC, SM)
                AL = smp.tile([128, N], f32, tag="al")
                nc.vector.tensor_scalar_mul(AL, EX, RC[:, 0:1])

                # ---- stores
                nc.sync.dma_start(out=alpha_o[128 * g:128 * (g + 1), :], in_=AL)
                nc.sync.dma_start(out=value_o[b0], in_=V2[0:64, :])
                nc.sync.dma_start(out=value_o[b1], in_=V2[64:128, :])
    nc.finalize()
    return nc


def _prep_host(inputs):
    emb = np.asarray(inputs["embeddings"], np.float32)       # [B, N, D]
    w_W = np.asarray(inputs["w_W"], np.float32)              # [e, d]
    w_b = np.asarray(inputs["w_b"], np.float32)              # [e]
    a_W = np.asarray(inputs["a_W"], np.float32)              # [e]

    pos = np.where(a_W >= 0)[0]
    neg = np.where(a_W < 0)[0]
    perm = np.concatenate([pos, neg])
    absa = np.abs(a_W[perm]) * (1.0 - NEG_SLOPE)

    wWT2 = np.zeros((65, N), np.float32)
    wWT2[:D, :] = w_W[perm].T * absa[None, :]
    wWT2[64, :] = w_b[perm] * absa

    wA = np.zeros((65, 1), np.float32)
    wA[:D, 0] = NEG_SLOPE * (w_W.T @ a_W)
    wA[64, 0] = NEG_SLOPE * float(a_W @ w_b)

    sel = np.zeros((2, 128), np.float32)
    sel[0, :64] = 1.0
    sel[1, 64:] = 1.0

    embT = np.empty((B, 65, N), np.float32)
    embT[:, :D, :] = emb.transpose(0, 2, 1)
    embT[:, 64, :] = 1.0
    embF = emb.reshape(B, N * D)
    return embT, embF, wWT2, wA, sel, len(pos)


def kernel(**inputs):
    from concourse.bass_utils import run_bass_kernel_spmd

    embT, embF, wWT2, wA, sel, pos_cnt = _prep_host(inputs)

    key = ("nc", pos_cnt)
    if key not in _CACHE:
        _CACHE[key] = _build_nc(pos_cnt)
    nc = _CACHE[key]

    in_maps = []
    for c in range(NCORES):
        s = slice(c * BS, (c + 1) * BS)
        in_maps.append({
            "embT": embT[s], "embF": embF[s],
            "wWT2": wWT2, "wA": wA, "sel": sel,
        })
    res = run_bass_kernel_spmd(nc, in_maps, core_ids=list(range(NCORES)))

    value = np.empty((B, N, N, D), np.float32)
    alphas = np.empty((B, N, N, 1), np.float32)
    for c in range(NCORES):
        r = res.results[c]
        value[c * BS:(c + 1) * BS] = r["value"].reshape(BS, N, N, D)
        alphas[c * BS:(c + 1) * BS] = r["alphas"].reshape(BS, N, N, 1)
    return alphas, value


# revision 7
# speedup vs baseline: 1.5870x; 1.5870x over previous
"""AFM sparse-attention kernel for 8 TRN2 NeuronCores.

Problem (per reference):
    value[b,i,j,:] = emb[b,i,:] * emb[b,j,:]                  [B,N,N,d]
    qk = LeakyReLU(value @ w_W.T + w_b, 0.01)
    logits = qk @ a_W (+ a_b, softmax-invariant)
    alphas = softmax(logits, axis=-1)[..., None]              [B,N,N,1]
    returns (alphas, value)

B=256, N=64, d=64.  Pure data parallel: batch sharded 32/core over 8 cores.

Device-side design per core (16 groups of 2 batches):
  - value (the 268MB output) is produced in DMA-native layout
    [i on partitions, (j,d) on free] as:  EF (emb[b] flattened,
    replicated across partitions via a K=2 selector matmul into PSUM)
    multiplied by a stride-0 broadcast view of emb rows (DVE).
    The per-batch store is one fully contiguous 1MB DMA.
  - qk[i,(j,e)] = ET_ext.T @ WJ where ET_ext = [emb[b].T ; ones] and
    WJ[d',(j,e)] = ET_ext[d',j] * wWT2_ext[d',e] (one DVE op per batch
    using stride-0 views; row 64 supplies the w_b bias).
  - LeakyReLU(y)*a = 0.01*a*y + 0.99*a*Relu(y); the linear term is an
    extra matmul (per-column scale c[d]), Relu runs on ACT, and the
    a_W-weighted e-contraction becomes two sign-grouped tensor_reduce
    ops (|a_W|*0.99 folded into the weights host-side).
  - softmax over j on [2*64 partitions, 64] tiles.
"""

import numpy as np

B, N, D = 256, 64, 64
NCORES = 8
BS = B // NCORES          # 32 batches per core
NG = BS // 2              # 16 two-batch groups
NEG_SLOPE = 0.01
F32 = None  # set after mybir import

_CACHE = {}


def _build_nc(pos_cnt):
    import concourse.bass as bass
    import concourse.bacc as bacc
    import concourse.mybir as mybir
    import concourse.tile as tile

    f32 = mybir.dt.float32
    nc = bacc.Bacc()

    bf16 = mybir.dt.bfloat16
    embT = nc.declare_dram_parameter("embT", [BS, 65, N], bf16, isOutput=False)
    embF = nc.declare_dram_parameter("embF", [BS, N * D], f32, isOutput=False)
    embHL = nc.declare_dram_parameter("embHL", [NG, 4, N * D], bf16, isOutput=False)
    wWT2 = nc.declare_dram_parameter("wWT2", [65, N], f32, isOutput=False)
    wA = nc.declare_dram_parameter("wA", [65, 1], f32, isOutput=False)
    sel = nc.declare_dram_parameter("sel", [4, 128], bf16, isOutput=False)
    value_o = nc.declare_dram_parameter("value", [BS, N, N * D], f32, isOutput=True)
    alpha_o = nc.declare_dram_parameter("alphas", [BS * N, N], f32, isOutput=True)

    Pp = pos_cnt  # columns 0..Pp-1 positive a_W, Pp..63 negative

    with tile.TileContext(nc) as tc:
        with (
            tc.tile_pool(name="consts", bufs=1) as consts,
            tc.tile_pool(name="inp", bufs=3) as inp,
            tc.tile_pool(name="wj", bufs=2) as wjp,
            tc.tile_pool(name="big", bufs=2) as big,
            tc.tile_pool(name="lq", bufs=2) as lqp,
            tc.tile_pool(name="sm", bufs=3) as smp,
            tc.tile_pool(name="efp", bufs=2, space="PSUM") as efpp,
            tc.tile_pool(name="qkp", bufs=3, space="PSUM") as qkpp,
            tc.tile_pool(name="ap", bufs=2, space="PSUM") as app,
        ):
            WT = consts.tile([65, N], f32)
            nc.sync.dma_start(out=WT, in_=wWT2[:])
            WA = consts.tile([65, 1], f32)
            nc.sync.dma_start(out=WA, in_=wA[:])
            SEL = consts.tile([4, 128], bf16)
            nc.sync.dma_start(out=SEL, in_=sel[:])

            wta = WT[:]
            # viewWT[d', (j, e)] = WT[d', e]   (j outer, stride 0)
            viewWT = bass.AP(tensor=wta.tensor, offset=wta.offset,
                             ap=[wta.ap[0], [0, N], [wta.ap[1][0], N]])

            for g in range(NG):
                b0, b1 = 2 * g, 2 * g + 1
                # ---- loads
                ET0 = inp.tile([65, N], bf16, tag="et0")
                nc.sync.dma_start(out=ET0, in_=embT[b0])
                ET1 = inp.tile([65, N], bf16, tag="et1")
                nc.sync.dma_start(out=ET1, in_=embT[b1])
                EMB2 = inp.tile([4, N * D], bf16, tag="emb2")
                nc.sync.dma_start(out=EMB2, in_=embHL[g])
                E128 = inp.tile([128, D], f32, tag="e128")
                nc.sync.dma_start(
                    out=E128,
                    in_=embF[b0:b0 + 2].rearrange("b (i d) -> (b i) d", d=D))

                # ---- WJ build (one [65, N*N] DVE op per batch)
                WJ0 = wjp.tile([65, N * N], bf16, tag="wj0")
                WJ1 = wjp.tile([65, N * N], bf16, tag="wj1")
                for ET, WJ in ((ET0, WJ0), (ET1, WJ1)):
                    eta = ET[:]
                    viewJ = bass.AP(tensor=eta.tensor, offset=eta.offset,
                                    ap=[eta.ap[0], [eta.ap[1][0], N], [0, N]])
                    nc.vector.tensor_mul(WJ, viewJ, viewWT)

                # WJA[d', j] = ET[d', j] * WA[d']  (A-term moving operand)
                WJA0 = inp.tile([65, N], bf16, tag="wja0")
                nc.vector.tensor_scalar_mul(WJA0, ET0, WA[:, 0:1])
                WJA1 = inp.tile([65, N], bf16, tag="wja1")
                nc.vector.tensor_scalar_mul(WJA1, ET1, WA[:, 0:1])

                # ---- A-term matmuls -> PSUM [128, 64]
                Aps = app.tile([128, N], f32, tag="aps")
                nc.tensor.matmul(Aps[0:64, :], ET0, WJA0, start=True, stop=True)
                nc.tensor.matmul(Aps[64:128, :], ET1, WJA1, start=True, stop=True,
                                 tile_position=(0, 64))

                # ---- value + qk chunk loop (8 chunks of 512)
                V2 = big.tile([128, N * D], f32, tag="v2")
                LQ = lqp.tile([128, N * N], f32, tag="lq")
                e1a = E128[:]
                for c in range(8):
                    sl = slice(512 * c, 512 * (c + 1))
                    # EF chunk: replicate emb2 rows across partition halves
                    EF = efpp.tile([128, 512], f32, tag="ef")
                    nc.tensor.matmul(EF, SEL, EMB2[:, sl], start=True, stop=True)
                    # value chunk: E128 row (stride-0 over j) * EF
                    viewE = bass.AP(
                        tensor=e1a.tensor, offset=e1a.offset,
                        ap=[e1a.ap[0], [0, 8], [e1a.ap[1][0], D]])
                    nc.vector.tensor_mul(V2[:, sl], EF, viewE)
                    # qk chunk for both batches -> one [128, 512] psum tile
                    QK = qkpp.tile([128, 512], f32, tag="qk")
                    nc.tensor.matmul(QK[0:64, :], ET0, WJ0[:, sl],
                                     start=True, stop=True)
                    nc.tensor.matmul(QK[64:128, :], ET1, WJ1[:, sl],
                                     start=True, stop=True, tile_position=(0, 64))
                    nc.scalar.activation(LQ[:, sl], QK,
                                         mybir.ActivationFunctionType.Relu)

                # ---- logits: sign-grouped reduces over e (inner dim)
                LG = smp.tile([128, N], f32, tag="lg")
                lqa = LQ[:]
                fs = lqa.ap[1][0]  # free step of LQ (elements)

                def lq_view(lo, cnt):
                    return bass.AP(tensor=lqa.tensor,
                                   offset=lqa.offset + fs * lo,
                                   ap=[lqa.ap[0], [fs * N, N], [fs, cnt]])

                if Pp > 0 and Pp < N:
                    RP = smp.tile([128, N], f32, tag="rp")
                    nc.vector.tensor_reduce(RP, lq_view(0, Pp),
                                            axis=mybir.AxisListType.X,
                                            op=mybir.AluOpType.add)
                    RN = smp.tile([128, N], f32, tag="rn")
                    nc.vector.tensor_reduce(RN, lq_view(Pp, N - Pp),
                                            axis=mybir.AxisListType.X,
                                            op=mybir.AluOpType.add, negate=True)
                    nc.vector.tensor_add(LG, RP, RN)
                else:
                    nc.vector.tensor_reduce(LG, lq_view(0, N),
                                            axis=mybir.AxisListType.X,
                                            op=mybir.AluOpType.add,
                                            negate=(Pp == 0))
                LG2 = smp.tile([128, N], f32, tag="lg2")
                nc.vector.tensor_add(LG2, LG, Aps)

                # ---- softmax over free dim (j)
                MX = smp.tile([128, 1], f32, tag="mx")
                nc.vector.tensor_reduce(MX, LG2, axis=mybir.AxisListType.X,
                                        op=mybir.AluOpType.max, negate=True)
                EX = smp.tile([128, N], f32, tag="ex")
                nc.scalar.activation(EX, LG2, mybir.ActivationFunctionType.Exp,
                                     bias=MX[:, 0:1], scale=1.0)
                SM = smp.tile([128, 1], f32, tag="sum")
                nc.vector.tensor_reduce(SM, EX, axis=mybir.AxisListType.X,
                                        op=mybir.AluOpType.add)
                RC = smp.tile([128, 1], f32, tag="rc")
                nc.vector.reciprocal(RC, SM)
                AL = smp.tile([128, N], f32, tag="al")
                nc.vector.tensor_scalar_mul(AL, EX, RC[:, 0:1])

                # ---- stores
                nc.sync.dma_start(out=alpha_o[128 * g:128 * (g + 1), :], in_=AL)
                nc.sync.dma_start(out=value_o[b0], in_=V2[0:64, :])
                nc.sync.dma_start(out=value_o[b1], in_=V2[64:128, :])
    nc.finalize()
    return nc


def _prep_host(inputs):
    emb = np.asarray(inputs["embeddings"], np.float32)       # [B, N, D]
    w_W = np.asarray(inputs["w_W"], np.float32)              # [e, d]
    w_b = np.asarray(inputs["w_b"], np.float32)              # [e]
    a_W = np.asarray(inputs["a_W"], np.float32)              # [e]

    pos = np.where(a_W >= 0)[0]
    neg = np.where(a_W < 0)[0]
    perm = np.concatenate([pos, neg])
    absa = np.abs(a_W[perm]) * (1.0 - NEG_SLOPE)

    wWT2 = np.zeros((65, N), np.float32)
    wWT2[:D, :] = w_W[perm].T * absa[None, :]
    wWT2[64, :] = w_b[perm] * absa

    wA = np.zeros((65, 1), np.float32)
    wA[:D, 0] = NEG_SLOPE * (w_W.T @ a_W)
    wA[64, 0] = NEG_SLOPE * float(a_W @ w_b)

    import ml_dtypes
    sel = np.zeros((4, 128), ml_dtypes.bfloat16)
    sel[0, :64] = 1.0
    sel[1, :64] = 1.0
    sel[2, 64:] = 1.0
    sel[3, 64:] = 1.0

    import ml_dtypes
    bf = ml_dtypes.bfloat16
    embT = np.empty((B, 65, N), bf)
    embT[:, :D, :] = emb.transpose(0, 2, 1).astype(bf)
    embT[:, 64, :] = 1.0
    embF = emb.reshape(B, N * D)
    hi = embF.astype(bf)
    lo = (embF - hi.astype(np.float32)).astype(bf)
    embHL = np.empty((B // 2, 4, N * D), bf)
    embHL[:, 0] = hi[0::2]
    embHL[:, 1] = lo[0::2]
    embHL[:, 2] = hi[1::2]
    embHL[:, 3] = lo[1::2]
    return embT, embF, embHL, wWT2, wA, sel, len(pos)


def _get_nc_and_maps(inputs):
    embT, embF, embHL, wWT2, wA, sel, pos_cnt = _prep_host(inputs)
    key = ("nc", pos_cnt)
    if key not in _CACHE:
        _CACHE[key] = _build_nc(pos_cnt)
    nc = _CACHE[key]
    in_maps = []
    for c in range(NCORES):
        s = slice(c * BS, (c + 1) * BS)
        in_maps.append({
            "embT": embT[s], "embF": embF[s],
            "embHL": embHL[c * NG:(c + 1) * NG],
            "wWT2": wWT2, "wA": wA, "sel": sel,
        })
    return nc, in_maps


def kernel(**inputs):
    from concourse.bass_utils import run_bass_kernel_spmd

    nc, in_maps = _get_nc_and_maps(inputs)
    res = run_bass_kernel_spmd(nc, in_maps, core_ids=list(range(NCORES)))

    value = np.empty((B, N, N, D), np.float32)
    alphas = np.empty((B, N, N, 1), np.float32)
    for c in range(NCORES):
        r = res.results[c]
        value[c * BS:(c + 1) * BS] = r["value"].reshape(BS, N, N, D)
        alphas[c * BS:(c + 1) * BS] = r["alphas"].reshape(BS, N, N, 1)
    return alphas, value
